# revision 28
# baseline (speedup 1.0000x reference)
"""Trainium2 Bass kernel for nn_BeamSearch (OPTW beam-search feasibility step).

Contract: kernel(**inputs) takes the FULL unsharded inputs and returns the
FULL output tuple (done, new_mask, adj, future_actions, present_time_new,
step_mask), matching the jax reference.

Strategy
--------
The only heavy output is adj [64, 1024, 1024] f32 (256 MB).  Everything else
is O(B*N) and is computed exactly on host in float32 numpy.

The reference adjacency is
    arr2  = dist[j,k] + fpresent[b,j]
    lhs   = arr2 + max(0, opening[b,j] - arr2)        (== max(arr2, open_bj))
    a1    = lhs <= closing[b,k]
    a2    = lhs + durat[b,k] + dlast[k] <= T_b
    adj   = new_mask[b,k] * a1 * a2 ;  diag = 1

Because IEEE f32 addition is monotone, {lhs : a2} == {lhs <= L2max[b,k]} where
L2max is the largest f32 x with ((x + durat) + dlast) <= T_b; we compute that
boundary exactly on host with nextafter fixups.  Folding the mask in as well:
    thr[b,k] = new_mask>0 ? min(closing, L2max) : -3e38
    adj[b,j,k] = (max(dist[j,k] + fp[b,j], open[b,j]) <= thr[b,k])
which was verified to reproduce the reference bit-exactly (0/64M flips).

Device kernel (per core, 8 beams):  dist stays resident in SBUF (4 MB);
per (beam, 128-row tile):
    t    = tensor_scalar(dist_tile, +fp[j], max open[j])   (DVE, 2x mode)
    adj  = tensor_tensor(t, thr_bcast, is_le)              (DVE, 1x mode)
    DMA out 512 KB
thr_bcast is one [128,1024] SWDGE partition-broadcast per beam (the 128x
re-read of one DRAM row is row-buffer-cheap; measured faster than GpSimd
partition_broadcast or on-chip doubling).  The diagonal is stitched in on
host (64K writes).

HW-measured on trn2 (For_i-looped timing, axon PJRT): ~120 us per core.
Rejected by measurement: GpSimd tensor_scalar offload (2-4x slower: shared
SBUF port contention + slow Q7 software ops), merge>1 TT grouping, HWDGE
sb2sb broadcast chains (ring FIFO blocks behind stores).

Sharding: beams 8*c .. 8*c+7 -> core c (data parallel, dist replicated).
"""

import numpy as np

B, N, P = 64, 1024, 128
NCORES = 8
BPC = B // NCORES   # beams per core
RT = N // P         # row tiles per beam

_CACHE = {}
LAST_RESULTS = None  # BassKernelResults of the most recent device run


def _build_bass(loops=None, merge=1, work_bufs=8, thr_bufs=2, thr_mode="swdge"):
    """loops=None: normal kernel.  loops=K: wrap the computation in a
    For_i(K) hardware loop — used only for benchmarking (device time is
    otherwise buried in the axon RPC floor).

    merge: row-tiles per tensor_tensor/store group (HW-measured best: 1;
    larger groups save DVE op overhead but the thr duplication and larger
    SWDGE descriptor chains cost more than they save)."""
    import concourse.bass as bass
    import concourse.bacc as bacc
    import concourse.mybir as mybir
    from concourse.tile import TileContext

    f32 = mybir.dt.float32
    nc = bacc.Bacc()
    dist = nc.declare_dram_parameter("dist", [N, N], f32, isOutput=False)
    # scal[:, 0:64] = fp cols, scal[:, 64:128] = open cols; col = i*RT + r
    scal = nc.declare_dram_parameter("scal", [P, 2 * BPC * RT], f32, isOutput=False)
    thr = nc.declare_dram_parameter("thr", [BPC, N], f32, isOutput=False)
    adj = nc.declare_dram_parameter("adj", [BPC, N, N], f32, isOutput=True)
    NCOL = BPC * RT

    with TileContext(nc) as tc:
        with (
            tc.tile_pool(name="dist", bufs=RT) as dpool,
            tc.tile_pool(name="const", bufs=1) as cpool,
            tc.tile_pool(name="thrp", bufs=thr_bufs) as thrpool,
            tc.tile_pool(name="probe", bufs=BPC) as prpool,
            tc.tile_pool(name="work", bufs=work_bufs) as work,
        ):
            dist_tiles = []
            for r in range(RT):
                dt_ = dpool.tile([P, N], f32, tag="dist_t")
                nc.sync.dma_start(out=dt_[:, :], in_=dist[r * P:(r + 1) * P, :])
                dist_tiles.append(dt_)
            scal_sb = cpool.tile([P, 2 * NCOL], f32)
            nc.sync.dma_start(out=scal_sb[:, :], in_=scal[:, :])

            def body():
                for i in range(BPC):
                    beam(i)

            def beam(i):
                # SWDGE partition-broadcast of thr[i], repeated `merge` times
                # along the free dim (the repeated DRAM row reads are
                # row-buffer hits, measured cheaper than on-chip alternatives)
                thr_bc = thrpool.tile([P, merge * N], f32)
                row = thr[i]
                bsrc = bass.AP(
                    tensor=row.tensor, offset=row.offset,
                    ap=[[0, P]] + list(row.ap),
                )
                nc.gpsimd.dma_start(out=thr_bc[:, 0:N], in_=bsrc)
                # duplicate along the free dim by on-chip doubling
                s = N
                while s < merge * N:
                    nc.gpsimd.dma_start(
                        out=thr_bc[:, s:2 * s], in_=thr_bc[:, 0:s]
                    )
                    s *= 2
                # Tiny DVE read of thr_bc: the TT instruction encoding has a
                # single sync-wait slot, so absorb the thr DMA wait here and
                # let the TTs below carry only their same-engine wait.
                pr = prpool.tile([P, 1], f32, tag="probe")
                nc.vector.tensor_copy(out=pr[:, :], in_=thr_bc[:, 0:1])
                for q in range(RT // merge):
                    t_tile = work.tile([P, merge * N], f32)
                    for u in range(merge):
                        r = q * merge + u
                        col = i * RT + r
                        nc.vector.tensor_scalar(
                            out=t_tile[:, u * N:(u + 1) * N],
                            in0=dist_tiles[r][:, :],
                            scalar1=scal_sb[:, col:col + 1],
                            scalar2=scal_sb[:, NCOL + col:NCOL + col + 1],
                            op0=mybir.AluOpType.add,
                            op1=mybir.AluOpType.max,
                        )
                    # one in-place compare for all `merge` row-tiles; the
                    # output-slot WAR wait stays on the tensor_scalars above
                    nc.vector.tensor_tensor(
                        out=t_tile[:, :],
                        in0=t_tile[:, :],
                        in1=thr_bc[:, :],
                        op=mybir.AluOpType.is_le,
                    )
                    # one store per group, alternating the two HWDGE rings
                    rows = adj[i, q * merge * P:(q + 1) * merge * P, :]
                    dst = rows.rearrange("(u p) k -> p u k", p=P)
                    src = t_tile[:, :].rearrange("p (u k) -> p u k", k=N)
                    dma_eng = nc.sync if (q % 2 == 0) else nc.scalar
                    dma_eng.dma_start(out=dst, in_=src)

            if loops is None:
                body()
            else:
                with tc.For_i(0, loops, 1):
                    body()
    nc.finalize()
    return nc


MERGE = 1   # row-tiles per tensor_tensor/store group (HW-measured best)


def _get_bass():
    if "nc" not in _CACHE:
        _CACHE["nc"] = _build_bass(merge=MERGE)
    return _CACHE["nc"]


def _host_prep(inputs, dist_mat, mask, present_time, pres_act):
    """Exact f32 replication of the reference's O(B*N) math."""
    f32 = np.float32
    x = np.asarray(inputs, dtype=f32)
    dist = np.asarray(dist_mat, dtype=f32)
    mask = np.asarray(mask, dtype=f32)
    pt = np.asarray(present_time, dtype=f32)
    pa = np.asarray(pres_act)
    bidx = np.arange(B)

    opening = x[..., 0]
    closing = x[..., 1]
    durat = x[..., 2]
    T_scalar = x[0, 0, 3]
    T_b = x[:, 0, 3]
    dlast = dist[:, -1]

    arrive = (dist[pa] + pt).astype(f32)
    wait = np.maximum(f32(0), opening - arrive)
    aw = arrive + wait
    c1 = aw <= closing
    c2 = ((aw + durat) + dlast[None, :]) <= T_scalar
    m = mask.copy()
    m[bidx, pa] = 0.0
    new_mask = (m * c1 * c2).astype(f32)
    done = not bool(np.any(new_mask[:, -1] > 0))

    fp = (aw + durat).astype(f32)

    # exact a2 boundary: largest f32 x with ((x + durat) + dlast) <= T_b
    d1 = durat
    d2 = np.broadcast_to(dlast[None, :], (B, N)).astype(f32)
    Tb = np.broadcast_to(T_b[:, None], (B, N)).astype(f32)

    def ok(v):
        return ((v + d1) + d2) <= Tb

    xx = ((Tb - d2) - d1).astype(f32)
    for _ in range(100):
        bad = ~ok(xx)
        if not bad.any():
            break
        xx = np.where(bad, np.nextafter(xx, -np.inf, dtype=f32), xx)
    for _ in range(100):
        nxt = np.nextafter(xx, np.inf, dtype=f32)
        up = ok(nxt)
        if not up.any():
            break
        xx = np.where(up, nxt, xx)

    thr = np.minimum(closing, xx).astype(f32)
    thr = np.where(new_mask > 0, thr, f32(-3e38)).astype(f32)
    return opening, new_mask, done, fp, thr


def kernel(inputs, dist_mat, mask, present_time, pres_act, future_actions):
    global LAST_RESULTS
    from concourse.bass_utils import run_bass_kernel_spmd

    opening, new_mask, done, fp, thr = _host_prep(
        inputs, dist_mat, mask, present_time, pres_act
    )

    dist_np = np.ascontiguousarray(np.asarray(dist_mat, dtype=np.float32))
    in_maps = []
    for c in range(NCORES):
        sl = slice(c * BPC, (c + 1) * BPC)
        # cols[p, i*RT + r] = v[beam i, r*P + p]
        fp_cols = fp[sl].reshape(BPC, RT, P).transpose(2, 0, 1).reshape(P, BPC * RT)
        open_cols = (
            opening[sl].reshape(BPC, RT, P).transpose(2, 0, 1).reshape(P, BPC * RT)
        )
        scal = np.ascontiguousarray(np.concatenate([fp_cols, open_cols], axis=1))
        in_maps.append(
            {
                "dist": dist_np,
                "scal": scal,
                "thr": np.ascontiguousarray(thr[sl]),
            }
        )

    nc = _get_bass()
    res = run_bass_kernel_spmd(nc, in_maps, list(range(NCORES)))
    LAST_RESULTS = res

    adj = np.concatenate([r["adj"] for r in res.results], axis=0)
    idx = np.arange(N)
    adj[:, idx, idx] = 1.0

    # small outputs (exact host replication of the reference)
    f32 = np.float32
    x = np.asarray(inputs, dtype=f32)
    dist = np.asarray(dist_mat, dtype=f32)
    pt = np.asarray(present_time, dtype=f32)
    pa = np.asarray(pres_act)
    fa = np.asarray(future_actions)
    bidx = np.arange(B)
    arrj = (dist[pa, fa][:, None] + pt).astype(f32)
    wj = np.maximum(f32(0), x[bidx, fa, 0][:, None] - arrj)
    present_time_new = (arrj + wj + x[bidx, fa, 2][:, None]).astype(f32)
    step_mask = np.ones((B, 1), dtype=bool)

    return (
        np.bool_(done),
        new_mask,
        adj,
        np.asarray(future_actions),
        present_time_new,
        step_mask,
    )
